# revision 21
# baseline (speedup 1.0000x reference)
"""Trainium2 Bass kernel for nn_ContextAttention (dense_transformer).

Reference model:
  neighbor_frames [2,2,96,96] -> per-frame conv3x3(1->64)+relu -> conv3x3(64->64)+relu
  feat [B, 128, 9216], Q/K/V 1x1 projections (64 out ch),
  attn = softmax(Q^T K / 8) [B, 9216, 9216], out = (attn @ V^T)^T -> [B, 64, 96, 96]

Distribution: 8 cores = 2 batches x 4 query-row blocks of 2304 tokens each.
Every core computes its batch's encoder feat (cheap vs attention) plus a
flash-style attention slice in S^T layout (key token m on partitions):

  R      = G^T feat_win + h        G = Wk Wq^T (feat basis), h = Wk q_b
  S^T    = feat_blk^T R            (PE, full K=128 contraction; q_b exact via h,
                                    k_b drops out of softmax exactly)
  expS   = exp(0.125 S^T - 4)      (ScalarE; softmax shift invariance)
  [ctx; rowsum] = sum_m expS[m,n] * VT1[m,:]   (PE; VT1 = [V^T | ones col])
  out    = ctx / rowsum + v_b

Both conv layers run the two frames as one block-diagonal matmul (f0 channels
on partitions 0-63, f1 on 64-127), so conv outputs land directly in the packed
feat layout with no cross-partition staging.  The whole kernel is software-
pipelined: attention query-chunk 0 (S/exp/AV) interleaves with the conv2/V
producer loop, group g consuming exactly the 3 m-blocks conv2 chunk g just
produced; AV lags one iteration so the PE never stalls on ScalarE's exp.

SPMD note: all 8 cores run one program; the only per-core knob is data.  The
per-core query window is realized by recomputing the two convs on a small
host-sliced window of the input image (xwin); out-of-image conv1 halo rows are
zeroed with a host mask to reproduce the reference's zero padding exactly.
"""

import numpy as np
import ml_dtypes

bf16 = ml_dtypes.bfloat16

B = 2
NF = 2
H = W = 96
HP = 98
T = H * W            # 9216
CH = 128
NB = T // 4          # 2304 query rows per core
NCORES = 8
MBLK = 128
NMB = T // MBLK      # 72
GROUP = 3            # m-blocks per exp group (3 PSUM banks)
CHUNKS = [512, 512, 512, 512, 256]
SHIFT = 4.0
SCALE = 0.125
WR1 = 26             # window conv1 output rows
WR2 = 24             # window conv2 output rows
RPC = 4              # image rows per conv chunk
NRC = H // RPC       # 24 full-image chunks; also the fused-loop length

_COMPILED = None
LAST_RESULTS = None


def _build_nc():
    import concourse.bass as bass
    import concourse.tile as tile
    from concourse import bacc, mybir

    f32 = mybir.dt.float32
    b16 = mybir.dt.bfloat16
    EXP = mybir.ActivationFunctionType.Exp
    ADD = mybir.AluOpType.add
    MAX = mybir.AluOpType.max

    nc = bacc.Bacc("TRN2", target_bir_lowering=False, debug=False,
                   enable_asserts=False, num_devices=NCORES)

    d_xpad = nc.dram_tensor("xpad", [NF, HP, HP], b16, kind="ExternalInput").ap()
    d_xwin = nc.dram_tensor("xwin", [NF, 28, HP], b16, kind="ExternalInput").ap()
    d_w1t = nc.dram_tensor("w1t", [18, CH], b16, kind="ExternalInput").ap()
    d_w2t = nc.dram_tensor("w2t", [CH, 9 * CH], b16, kind="ExternalInput").ap()
    d_gwt = nc.dram_tensor("gwt", [CH, CH], b16, kind="ExternalInput").ap()
    d_vwt = nc.dram_tensor("vwt", [CH, 64], b16, kind="ExternalInput").ap()
    d_b1 = nc.dram_tensor("b1", [CH, 1], f32, kind="ExternalInput").ap()
    d_b2 = nc.dram_tensor("b2", [CH, 1], f32, kind="ExternalInput").ap()
    d_hb = nc.dram_tensor("hb", [CH, 1], f32, kind="ExternalInput").ap()
    d_vb = nc.dram_tensor("vb", [64, 1], f32, kind="ExternalInput").ap()
    d_rmsk = nc.dram_tensor("rmsk", [CH, 2 * HP], b16, kind="ExternalInput").ap()
    d_out = nc.dram_tensor("out", [64, NB], f32, kind="ExternalOutput").ap()

    with tile.TileContext(nc) as tc:
        _frees = []

        def _keep(pair):
            _frees.append(pair[1])
            return pair[0]

        w1t = _keep(tc.tile([18, CH], b16, name="w1t_sb"))
        w2t = _keep(tc.tile([CH, 9 * CH], b16, name="w2t_sb"))
        gwt = _keep(tc.tile([CH, CH], b16, name="gwt_sb"))
        vwt = _keep(tc.tile([CH, 64], b16, name="vwt_sb"))
        b1s = _keep(tc.tile([CH, 1], f32, name="b1_sb"))
        b2s = _keep(tc.tile([CH, 1], f32, name="b2_sb"))
        hbs = _keep(tc.tile([CH, 1], f32, name="hb_sb"))
        vbs = _keep(tc.tile([64, 1], f32, name="vb_sb"))
        rmsk = _keep(tc.tile([CH, 2 * HP], b16, name="rmsk_sb"))
        x9 = _keep(tc.tile([18, T], b16, name="x9_sb"))          # f0 @ p0-8, f1 @ p9-17
        x9w = _keep(tc.tile([18, WR1 * W], b16, name="x9w_sb"))
        r1p = _keep(tc.tile([CH, HP * HP], b16, name="r1pad_sb"))  # f0 @ p0-63, f1 @ p64-127
        r1wp = _keep(tc.tile([CH, WR1 * HP], b16, name="r1wp_sb"))
        feat = _keep(tc.tile([CH, T], b16, name="feat_sb"))
        featw = _keep(tc.tile([CH, NB], b16, name="featw_sb"))
        rsb = _keep(tc.tile([CH, NB], b16, name="r_sb"))
        vt1 = _keep(tc.tile([CH, NMB * 65], b16, name="vt1_sb"))
        outs = _keep(tc.tile([64, NB], f32, name="out_sb"))
        shf = _keep(tc.tile([CH, 1], f32, name="shift_sb"))
        ones65 = _keep(tc.tile([65, 64], f32, name="ones65_sb"))

        dma = nc.sync.dma_start
        dma2 = nc.scalar.dma_start   # second DGE queue (scalar idle in prolog)

        # ---- load inputs: conv1w's deps split across both queues, rest after ----
        dma(w1t[:, :], d_w1t)
        dma(b1s[:, :], d_b1)
        for t in range(9):
            dy, dx = t // 3, t % 3
            dma(x9w[t:t + 1, :].rearrange("p (h w) -> p h w", h=WR1),
                d_xwin[0:1, dy:dy + WR1, dx:dx + W])
            dma2(x9w[9 + t:10 + t, :].rearrange("p (h w) -> p h w", h=WR1),
                 d_xwin[1:2, dy:dy + WR1, dx:dx + W])
        dma2(rmsk[:, :], d_rmsk)
        dma(w2t[:, :], d_w2t)
        dma(b2s[:, :], d_b2)
        dma(gwt[:, :], d_gwt)
        dma(hbs[:, :], d_hb)
        dma(vwt[:, :], d_vwt)
        dma(vbs[:, :], d_vb)
        for t in range(9):
            dy, dx = t // 3, t % 3
            dma(x9[t:t + 1, :].rearrange("p (h w) -> p h w", h=H),
                d_xpad[0:1, dy:dy + H, dx:dx + W])
            dma2(x9[9 + t:10 + t, :].rearrange("p (h w) -> p h w", h=H),
                 d_xpad[1:2, dy:dy + H, dx:dx + W])

        nc.vector.memset(shf[:, :], -SHIFT)
        nc.vector.memset(ones65[:, :], 1.0)
        # padded-conv1-output borders (interiors fully written below)
        r1v = r1p[:, :].rearrange("p (h w) -> p h w", h=HP)
        nc.vector.memset(r1v[:, 0:1, :], 0.0)
        nc.vector.memset(r1v[:, HP - 1:HP, :], 0.0)
        nc.vector.memset(r1v[:, :, 0:1], 0.0)
        nc.vector.memset(r1v[:, :, HP - 1:HP], 0.0)
        r1wv = r1wp[:, :].rearrange("p (h w) -> p h w", h=WR1)
        nc.vector.memset(r1wv[:, :, 0:1], 0.0)
        nc.vector.memset(r1wv[:, :, HP - 1:HP], 0.0)
        vt1v = vt1[:, :].rearrange("p (t c) -> p t c", c=65)
        nc.vector.memset(vt1v[:, :, 64:65], 1.0)

        x9v = x9[:, :].rearrange("p (h w) -> p h w", h=H)
        x9wv = x9w[:, :].rearrange("p (h w) -> p h w", h=WR1)
        featv = feat[:, :]
        rmv = rmsk[:, :].rearrange("p (h w) -> p h w", h=2)
        QCH = [(0, 512), (512, 512), (1024, 512), (1536, 512), (2048, 256)]

        # ctx pool (2 banks) spans all phases so chunk normalizes can defer
        # their PE/store half into the next chunk's pipeline.
        outer_ctx = tc.tile_pool(name="ctxp", bufs=2, space="PSUM")
        c_pool = outer_ctx.__enter__()
        outer_rn = tc.tile_pool(name="rnp", bufs=2)
        rn_pool = outer_rn.__enter__()
        outer_ex = tc.tile_pool(name="expp", bufs=3)
        e_pool = outer_ex.__enter__()

        # ---- prolog: window path (-> R) + full-image conv1 ----
        with tc.tile_pool(name="pw1", bufs=1, space="PSUM") as pw1, \
             tc.tile_pool(name="pw2", bufs=2, space="PSUM") as pw2, \
             tc.tile_pool(name="pc1", bufs=2, space="PSUM") as pc1, \
             tc.tile_pool(name="pr", bufs=1, space="PSUM") as pr:
            # window conv1 (both frames, block-diag K=18)
            for (r0, nr) in [(0, 4), (4, 4), (8, 4), (12, 4), (16, 4), (20, 4), (24, 2)]:
                ps = pw1.tile([CH, RPC * W], f32, tag="c1w")
                nc.tensor.matmul(ps[:, 0:nr * W], lhsT=w1t[:, :],
                                 rhs=x9wv[:, r0:r0 + nr, :], start=True, stop=True)
                nc.vector.tensor_scalar(
                    r1wv[:, r0:r0 + nr, 1:97],
                    ps[:, 0:nr * W].rearrange("p (h w) -> p h w", h=nr),
                    b1s[:, :], 0.0, op0=ADD, op1=MAX)
            # halo-row masks (rows 0 and WR1-1 only)
            nc.vector.tensor_mul(r1wv[:, 0:1, :], r1wv[:, 0:1, :], rmv[:, 0:1, :])
            nc.vector.tensor_mul(r1wv[:, WR1 - 1:WR1, :], r1wv[:, WR1 - 1:WR1, :],
                                 rmv[:, 1:2, :])
            # window conv2 -> featw
            for c in range(WR2 // RPC):
                ps = pw2.tile([CH, RPC * W], f32, tag="c2w")
                for t in range(9):
                    dy, dx = t // 3, t % 3
                    nc.tensor.matmul(
                        ps[:, :], lhsT=w2t[:, bass.ts(t, CH)],
                        rhs=r1wv[:, dy + c * RPC:dy + (c + 1) * RPC, dx:dx + W],
                        start=(t == 0), stop=(t == 8))
                nc.vector.tensor_scalar(featw[:, bass.ts(c, RPC * W)], ps[:, :],
                                        b2s[:, :], 0.0, op0=ADD, op1=MAX)
            # R = G^T featw + h
            for (q0, qn) in QCH:
                ps = pr.tile([CH, 512], f32, tag="r")
                nc.tensor.matmul(ps[:, 0:qn], lhsT=gwt[:, :],
                                 rhs=featw[:, bass.ds(q0, qn)], start=True, stop=True)
                nc.vector.tensor_scalar(rsb[:, bass.ds(q0, qn)], ps[:, 0:qn],
                                        hbs[:, :], None, op0=ADD)
            # full-image conv1 -> r1p interior
            for c in range(NRC):
                r0 = RPC * c
                ps = pc1.tile([CH, RPC * W], f32, tag="c1f")
                nc.tensor.matmul(ps[:, :], lhsT=w1t[:, :],
                                 rhs=x9v[:, r0:r0 + RPC, :], start=True, stop=True)
                nc.vector.tensor_scalar(
                    r1v[:, r0 + 1:r0 + RPC + 1, 1:97],
                    ps[:, :].rearrange("p (h w) -> p h w", h=RPC),
                    b1s[:, :], 0.0, op0=ADD, op1=MAX)

        def s_group(sp, g, n_off, nch):
            for j in range(GROUP):
                m = GROUP * g + j
                nc.tensor.matmul(sp[:, j * 512:j * 512 + nch],
                                 lhsT=feat[:, bass.ts(m, MBLK)],
                                 rhs=rsb[:, bass.ds(n_off, nch)],
                                 start=True, stop=True)

        def exp_group(ex, sp, nch):
            if nch == 512:
                nc.scalar.activation(ex[:, :], sp[:, :], EXP, bias=shf[:, :],
                                     scale=SCALE)
            else:
                nc.scalar.activation(
                    ex[:, :].rearrange("p (j x) -> p j x", x=512)[:, :, 0:nch],
                    sp[:, :].rearrange("p (j x) -> p j x", x=512)[:, :, 0:nch],
                    EXP, bias=shf[:, :], scale=SCALE)

        def av_group(ctx_ps, ex, g, nch):
            for j in range(GROUP):
                m = GROUP * g + j
                nc.tensor.matmul(ctx_ps[:, 0:nch],
                                 lhsT=vt1[:, bass.ts(m, 65)],
                                 rhs=ex[:, j * 512:j * 512 + nch],
                                 start=(m == 0), stop=(m == NMB - 1))

        def norm_dve(ctx_ps, nch):
            # copy ctx out of PSUM fast (frees the bank for the next chunk)
            # and recip the rowsum; DVE only, so the PE queue never waits.
            ctxs = rn_pool.tile([65, 512], f32, tag="ctxs")
            nc.vector.tensor_copy(ctxs[:, 0:nch], ctx_ps[:, 0:nch])
            rr = rn_pool.tile([65, 512], f32, tag="rr")
            nc.vector.reciprocal(rr[64:65, 0:nch], ctxs[64:65, 0:nch])
            return ctxs, rr

        def norm_pe(ctxs, rr, n_off, nch):
            # broadcast 1/rowsum across partitions with a K=1 fp32 outer
            # product into a free ctx-pool slot, then scale + v_b + store.
            bc = c_pool.tile([65, 512], f32, tag="ctx")
            nc.tensor.matmul(bc[0:64, 0:nch], lhsT=ones65[64:65, 0:64],
                             rhs=rr[64:65, 0:nch], start=True, stop=True,
                             tile_position=(64, 0))
            nc.vector.tensor_mul(outs[:, bass.ds(n_off, nch)],
                                 ctxs[0:64, 0:nch], bc[0:64, 0:nch])
            nc.vector.tensor_scalar(outs[:, bass.ds(n_off, nch)],
                                    outs[:, bass.ds(n_off, nch)],
                                    vbs[:, :], None, op0=ADD)
            dma(d_out[:, bass.ds(n_off, nch)], outs[:, bass.ds(n_off, nch)])

        # ---- fused producer + attention chunk 0 ----
        # iteration c: conv2 chunk c -> feat, V^T blocks, AV(c-1), S(c), exp(c)
        ctx0 = c_pool.tile([65, 512], f32, tag="ctx", name="ctx0")
        with tc.tile_pool(name="pc2", bufs=2, space="PSUM") as pc2, \
             tc.tile_pool(name="pv", bufs=1, space="PSUM") as pv, \
             tc.tile_pool(name="sp0", bufs=1, space="PSUM") as sp0_pool:
            prev = None
            for c in range(NRC):
                ps = pc2.tile([CH, RPC * W], f32, tag="c2f")
                for t in range(9):
                    dy, dx = t // 3, t % 3
                    nc.tensor.matmul(
                        ps[:, :], lhsT=w2t[:, bass.ts(t, CH)],
                        rhs=r1v[:, dy + c * RPC:dy + (c + 1) * RPC, dx:dx + W],
                        start=(t == 0), stop=(t == 8))
                nc.vector.tensor_scalar(featv[:, bass.ts(c, RPC * W)], ps[:, :],
                                        b2s[:, :], 0.0, op0=ADD, op1=MAX)
                psv = pv.tile([CH, GROUP * 64], f32, tag="v")
                for j in range(GROUP):
                    m = GROUP * c + j
                    nc.tensor.matmul(psv[:, bass.ts(j, 64)],
                                     lhsT=feat[:, bass.ts(m, MBLK)],
                                     rhs=vwt[:, :], start=True, stop=True)
                nc.vector.tensor_copy(
                    vt1v[:, GROUP * c:GROUP * (c + 1), 0:64],
                    psv[:, :].rearrange("p (t c) -> p t c", c=64))
                if prev is not None:
                    av_group(ctx0, prev[0], prev[1], 512)
                sp = sp0_pool.tile([CH, GROUP * 512], f32, tag="sp0")
                s_group(sp, c, 0, 512)
                ex = e_pool.tile([CH, GROUP * 512], b16, tag="ex0")
                exp_group(ex, sp, 512)
                prev = (ex, c)
            # last AV group + normalize defer into the next phase's pipeline
            pend_av = (ctx0, prev[0], prev[1], 512, 0)

        # ---- remaining query chunks ----
        with tc.tile_pool(name="spp", bufs=2, space="PSUM") as sp_pool:
            pend_norm = None
            n_off = 512
            for nch in CHUNKS[1:]:
                ctx_ps = c_pool.tile([65, 512], f32, tag="ctx")
                prev = None
                for g in range(NMB // GROUP):
                    if prev is not None:
                        av_group(ctx_ps, prev[0], prev[1], nch)
                    sp = sp_pool.tile([CH, GROUP * 512], f32, tag="sp")
                    s_group(sp, g, n_off, nch)
                    ex = e_pool.tile([CH, GROUP * 512], b16, tag="ex")
                    exp_group(ex, sp, nch)
                    if g == 0 and pend_av is not None:
                        pctx, pex, pg, pnch, poff = pend_av
                        av_group(pctx, pex, pg, pnch)
                        pend_norm = norm_dve(pctx, pnch) + (poff, pnch)
                        pend_av = None
                    if g == 2 and pend_norm is not None:
                        norm_pe(*pend_norm)
                        pend_norm = None
                    prev = (ex, g)
                pend_av = (ctx_ps, prev[0], prev[1], nch, n_off)
                n_off += nch
            pctx, pex, pg, pnch, poff = pend_av
            av_group(pctx, pex, pg, pnch)
            norm_pe(*(norm_dve(pctx, pnch) + (poff, pnch)))

        outer_ex.__exit__(None, None, None)
        outer_rn.__exit__(None, None, None)
        outer_ctx.__exit__(None, None, None)

        for _f in reversed(_frees):
            _f()

    nc.compile()
    return nc


def _prep_in_maps(inputs):
    x = np.asarray(inputs["neighbor_frames"], np.float32)
    w1 = np.asarray(inputs["enc_w1"], np.float32)
    w2 = np.asarray(inputs["enc_w2"], np.float32)
    qw = np.asarray(inputs["q_w"], np.float32)
    kw = np.asarray(inputs["k_w"], np.float32)
    vw = np.asarray(inputs["v_w"], np.float32)
    b1 = np.asarray(inputs["enc_b1"], np.float32)
    b2 = np.asarray(inputs["enc_b2"], np.float32)
    qb = np.asarray(inputs["q_b"], np.float32)
    vb = np.asarray(inputs["v_b"], np.float32)
    # k_b intentionally unused: it shifts every logit of a query row by the
    # same constant, which softmax cancels exactly.

    xpad = np.zeros((B, NF, HP, HP), np.float32)
    xpad[:, :, 1:97, 1:97] = x

    # per-core query windows: input rows r0-2 .. r0+25 (zero outside image)
    xbig = np.zeros((B, NF, 102, HP), np.float32)   # row i = input row i-3
    xbig[:, :, 3:99, 1:97] = x
    xwin = np.zeros((B, 4, NF, 28, HP), np.float32)
    for q in range(4):
        r0 = q * 24
        xwin[:, q] = xbig[:, :, r0 + 1:r0 + 29, :]
    xwin = xwin.astype(bf16)
    xpad = xpad.astype(bf16)

    # conv1w halo-row masks: plane 0 = window row 0, plane 1 = window row WR1-1
    rmasks = []
    for q in range(4):
        m = np.ones((CH, 2, HP), np.float32)
        if q == 0:
            m[:, 0, :] = 0.0
        if q == 3:
            m[:, 1, :] = 0.0
        rmasks.append(np.ascontiguousarray(m.reshape(CH, 2 * HP)).astype(bf16))

    # conv1 weights, block-diagonal: taps f0 on rows 0-8 -> cols 0-63,
    # taps f1 on rows 9-17 -> cols 64-127 (same weights, frames share encoder)
    taps = w1.reshape(64, 9).T                     # [9, 64]
    w1t = np.zeros((18, CH), np.float32)
    w1t[0:9, 0:64] = taps
    w1t[9:18, 64:128] = taps
    w1t = w1t.astype(bf16)

    # conv2 block-diagonal per tap: [128 cin, tap*128 + cout]
    w2tap = w2.transpose(2, 3, 1, 0).reshape(9, 64, 64)  # [tap, cin, cout]
    w2t = np.zeros((CH, 9 * CH), np.float32)
    for t in range(9):
        w2t[0:64, t * CH:t * CH + 64] = w2tap[t]
        w2t[64:128, t * CH + 64:t * CH + 128] = w2tap[t]
    w2t = w2t.astype(bf16)

    # feat block layout row i = frame*64 + c  <->  reference channel c*2+frame
    perm = np.array([(i % 64) * 2 + i // 64 for i in range(CH)])
    qwt = np.ascontiguousarray(qw[:, perm].T)      # [128, 64]
    kwt = np.ascontiguousarray(kw[:, perm].T)
    vwt = np.ascontiguousarray(vw[:, perm].T).astype(bf16)
    gwt = np.ascontiguousarray(qwt @ kwt.T).astype(bf16)   # lhsT of R = G^T featw
    hb = np.ascontiguousarray((kwt @ qb).reshape(CH, 1))   # [128, 1] f32

    b1c = np.ascontiguousarray(np.concatenate([b1, b1]).reshape(CH, 1))
    b2c = np.ascontiguousarray(np.concatenate([b2, b2]).reshape(CH, 1))
    vbc = np.ascontiguousarray(vb.reshape(64, 1))

    in_maps = []
    for core in range(NCORES):
        b = core // 4
        q = core % 4
        in_maps.append({
            "xpad": np.ascontiguousarray(xpad[b]),
            "xwin": np.ascontiguousarray(xwin[b, q]),
            "rmsk": rmasks[q],
            "w1t": w1t, "w2t": w2t, "gwt": gwt, "vwt": vwt,
            "b1": b1c, "b2": b2c, "hb": hb, "vb": vbc,
        })
    return in_maps


def _install_ntff_shim():
    """Provide antenv.axon_hooks (absent in this image) so
    run_bass_kernel_spmd(trace=True) can capture NTFF profiles through
    libaxon_pjrt's C ABI, and neuter the S3 artifact upload."""
    import sys, types, ctypes, contextlib

    if "antenv.axon_hooks" not in sys.modules:
        mod = types.ModuleType("antenv.axon_hooks")
        mod._hook = None
        mod.set_axon_ntff_profile_hook = lambda h: setattr(mod, "_hook", h)
        mod.get_axon_ntff_profile_hook = lambda: mod._hook
        sys.modules["antenv.axon_hooks"] = mod

        lib = ctypes.CDLL("/opt/axon/libaxon_pjrt.so")
        if hasattr(lib, "axon_start_nrt_profile"):
            lib.axon_start_nrt_profile.argtypes = [
                ctypes.POINTER(ctypes.c_int64), ctypes.c_size_t]
            lib.axon_start_nrt_profile.restype = ctypes.c_int64
            lib.axon_stop_nrt_profile.argtypes = [ctypes.c_char_p]
            lib.axon_stop_nrt_profile.restype = ctypes.c_int64

            @contextlib.contextmanager
            def _hook(output_dir, device_ids):
                import jax
                jax.devices()
                if device_ids:
                    ids = (ctypes.c_int64 * len(device_ids))(*device_ids)
                    rc = lib.axon_start_nrt_profile(ids, len(device_ids))
                else:
                    rc = lib.axon_start_nrt_profile(None, 0)
                if rc != 0:
                    raise RuntimeError(f"axon_start_nrt_profile rc={rc}")
                try:
                    yield
                finally:
                    n = lib.axon_stop_nrt_profile(str(output_dir).encode())
                    print(f"ntff profile: {n} file(s) -> {output_dir}")

            mod.set_axon_ntff_profile_hook(_hook)

    import concourse.bass_utils as _bu
    _bu.upload_artifacts = lambda tmpdir: tmpdir


def kernel(**inputs):
    global _COMPILED, LAST_RESULTS
    from concourse.bass_utils import run_bass_kernel_spmd

    if _COMPILED is None:
        _COMPILED = _build_nc()
    nc = _COMPILED

    in_maps = _prep_in_maps(inputs)
    trace = bool(int(__import__("os").environ.get("CA_TRACE", "0")))
    if trace:
        _install_ntff_shim()
    res = run_bass_kernel_spmd(nc, in_maps, core_ids=list(range(NCORES)),
                               trace=trace)
    LAST_RESULTS = res

    out = np.zeros((B, 64, T), np.float32)
    for core in range(NCORES):
        b = core // 4
        q = core % 4
        out[b, :, q * NB:(q + 1) * NB] = res.results[core]["out"]
    return out.reshape(B, 64, H, W)


# revision 22
# speedup vs baseline: 1.2032x; 1.2032x over previous
"""Trainium2 Bass kernel for nn_ContextAttention (dense_transformer).

Reference model:
  neighbor_frames [2,2,96,96] -> per-frame conv3x3(1->64)+relu -> conv3x3(64->64)+relu
  feat [B, 128, 9216], Q/K/V 1x1 projections (64 out ch),
  attn = softmax(Q^T K / 8) [B, 9216, 9216], out = (attn @ V^T)^T -> [B, 64, 96, 96]

Distribution: 8 cores = 2 batches x 4 query-row blocks of 2304 tokens each.
Every core computes its batch's encoder feat (cheap vs attention) plus a
flash-style attention slice in S^T layout (key token m on partitions):

  R      = G^T feat_win + h        G = Wk Wq^T (feat basis), h = Wk q_b
  S^T    = feat_blk^T R            (PE, full K=128 contraction; q_b exact via h,
                                    k_b drops out of softmax exactly)
  expS   = exp(0.125 S^T - 4)      (ScalarE; softmax shift invariance)
  [ctx; rowsum] = sum_m expS[m,n] * VT1[m,:]   (PE; VT1 = [V^T | ones col])
  out    = ctx / rowsum + v_b

Both conv layers run the two frames as one block-diagonal matmul (f0 channels
on partitions 0-63, f1 on 64-127), so conv outputs land directly in the packed
feat layout with no cross-partition staging.  The whole kernel is software-
pipelined: attention query-chunk 0 (S/exp/AV) interleaves with the conv2/V
producer loop, group g consuming exactly the 3 m-blocks conv2 chunk g just
produced; AV lags one iteration so the PE never stalls on ScalarE's exp.

SPMD note: all 8 cores run one program; the only per-core knob is data.  The
per-core query window is realized by recomputing the two convs on a small
host-sliced window of the input image (xwin); out-of-image conv1 halo rows are
zeroed with a host mask to reproduce the reference's zero padding exactly.
"""

import numpy as np
import ml_dtypes

bf16 = ml_dtypes.bfloat16

B = 2
NF = 2
H = W = 96
HP = 98
T = H * W            # 9216
CH = 128
NB = T // 4          # 2304 query rows per core
NCORES = 8
MBLK = 128
NMB = T // MBLK      # 72
GROUP = 3            # m-blocks per exp group (3 PSUM banks)
CHUNKS = [512, 512, 512, 512, 256]
SHIFT = 4.0
SCALE = 0.125
WR1 = 26             # window conv1 output rows
WR2 = 24             # window conv2 output rows
RPC = 4              # image rows per conv chunk
NRC = H // RPC       # 24 full-image chunks; also the fused-loop length

_COMPILED = None
LAST_RESULTS = None


def _build_nc():
    import concourse.bass as bass
    import concourse.tile as tile
    from concourse import bacc, mybir

    f32 = mybir.dt.float32
    b16 = mybir.dt.bfloat16
    EXP = mybir.ActivationFunctionType.Exp
    ADD = mybir.AluOpType.add
    MAX = mybir.AluOpType.max

    nc = bacc.Bacc("TRN2", target_bir_lowering=False, debug=False,
                   enable_asserts=False, num_devices=NCORES)

    d_xpad = nc.dram_tensor("xpad", [NF, HP, HP], b16, kind="ExternalInput").ap()
    d_xwin = nc.dram_tensor("xwin", [NF, 28, HP], b16, kind="ExternalInput").ap()
    d_w1t = nc.dram_tensor("w1t", [18, CH], b16, kind="ExternalInput").ap()
    d_w2t = nc.dram_tensor("w2t", [CH, 9 * CH], b16, kind="ExternalInput").ap()
    d_gwt = nc.dram_tensor("gwt", [CH, CH], b16, kind="ExternalInput").ap()
    d_vwt = nc.dram_tensor("vwt", [CH, 64], b16, kind="ExternalInput").ap()
    d_b1 = nc.dram_tensor("b1", [CH, 1], f32, kind="ExternalInput").ap()
    d_b2 = nc.dram_tensor("b2", [CH, 1], f32, kind="ExternalInput").ap()
    d_hb = nc.dram_tensor("hb", [CH, 1], f32, kind="ExternalInput").ap()
    d_vb = nc.dram_tensor("vb", [64, 1], f32, kind="ExternalInput").ap()
    d_rmsk = nc.dram_tensor("rmsk", [CH, 2 * HP], b16, kind="ExternalInput").ap()
    d_out = nc.dram_tensor("out", [64, NB], f32, kind="ExternalOutput").ap()

    with tile.TileContext(nc) as tc:
        _frees = []

        def _keep(pair):
            _frees.append(pair[1])
            return pair[0]

        w1t = _keep(tc.tile([18, CH], b16, name="w1t_sb"))
        w2t = _keep(tc.tile([CH, 9 * CH], b16, name="w2t_sb"))
        gwt = _keep(tc.tile([CH, CH], b16, name="gwt_sb"))
        vwt = _keep(tc.tile([CH, 64], b16, name="vwt_sb"))
        b1s = _keep(tc.tile([CH, 1], f32, name="b1_sb"))
        b2s = _keep(tc.tile([CH, 1], f32, name="b2_sb"))
        hbs = _keep(tc.tile([CH, 1], f32, name="hb_sb"))
        vbs = _keep(tc.tile([64, 1], f32, name="vb_sb"))
        rmsk = _keep(tc.tile([CH, 2 * HP], b16, name="rmsk_sb"))
        x9 = _keep(tc.tile([18, T], b16, name="x9_sb"))          # f0 @ p0-8, f1 @ p9-17
        x9w = _keep(tc.tile([18, WR1 * W], b16, name="x9w_sb"))
        r1p = _keep(tc.tile([CH, HP * HP], b16, name="r1pad_sb"))  # f0 @ p0-63, f1 @ p64-127
        r1wp = _keep(tc.tile([CH, WR1 * HP], b16, name="r1wp_sb"))
        feat = _keep(tc.tile([CH, T], b16, name="feat_sb"))
        featw = _keep(tc.tile([CH, NB], b16, name="featw_sb"))
        rsb = _keep(tc.tile([CH, NB], b16, name="r_sb"))
        vt1 = _keep(tc.tile([CH, NMB * 65], b16, name="vt1_sb"))
        outs = _keep(tc.tile([64, NB], f32, name="out_sb"))
        shf = _keep(tc.tile([CH, 1], f32, name="shift_sb"))
        ones65 = _keep(tc.tile([65, 64], f32, name="ones65_sb"))

        dma = nc.sync.dma_start
        dma2 = nc.scalar.dma_start   # second DGE queue (scalar idle in prolog)

        # ---- load inputs: conv1w's deps split across both queues, rest after ----
        dma(w1t[:, :], d_w1t)
        dma(b1s[:, :], d_b1)
        for t in range(9):
            dy, dx = t // 3, t % 3
            dma(x9w[t:t + 1, :].rearrange("p (h w) -> p h w", h=WR1),
                d_xwin[0:1, dy:dy + WR1, dx:dx + W])
            dma2(x9w[9 + t:10 + t, :].rearrange("p (h w) -> p h w", h=WR1),
                 d_xwin[1:2, dy:dy + WR1, dx:dx + W])
        dma2(rmsk[:, :], d_rmsk)
        dma(w2t[:, :], d_w2t)
        dma(b2s[:, :], d_b2)
        dma(gwt[:, :], d_gwt)
        dma(hbs[:, :], d_hb)
        dma(vwt[:, :], d_vwt)
        dma(vbs[:, :], d_vb)
        for t in range(9):
            dy, dx = t // 3, t % 3
            dma(x9[t:t + 1, :].rearrange("p (h w) -> p h w", h=H),
                d_xpad[0:1, dy:dy + H, dx:dx + W])
            dma2(x9[9 + t:10 + t, :].rearrange("p (h w) -> p h w", h=H),
                 d_xpad[1:2, dy:dy + H, dx:dx + W])

        nc.vector.memset(shf[:, :], -SHIFT)
        nc.vector.memset(ones65[:, :], 1.0)
        # padded-conv1-output borders (interiors fully written below)
        r1v = r1p[:, :].rearrange("p (h w) -> p h w", h=HP)
        nc.vector.memset(r1v[:, 0:1, :], 0.0)
        nc.vector.memset(r1v[:, HP - 1:HP, :], 0.0)
        nc.vector.memset(r1v[:, :, 0:1], 0.0)
        nc.vector.memset(r1v[:, :, HP - 1:HP], 0.0)
        r1wv = r1wp[:, :].rearrange("p (h w) -> p h w", h=WR1)
        nc.vector.memset(r1wv[:, :, 0:1], 0.0)
        nc.vector.memset(r1wv[:, :, HP - 1:HP], 0.0)
        vt1v = vt1[:, :].rearrange("p (t c) -> p t c", c=65)
        nc.vector.memset(vt1v[:, :, 64:65], 1.0)

        x9v = x9[:, :].rearrange("p (h w) -> p h w", h=H)
        x9wv = x9w[:, :].rearrange("p (h w) -> p h w", h=WR1)
        featv = feat[:, :]
        rmv = rmsk[:, :].rearrange("p (h w) -> p h w", h=2)
        QCH = [(0, 512), (512, 512), (1024, 512), (1536, 512), (2048, 256)]

        # ctx pool (2 banks) spans all phases so chunk normalizes can defer
        # their PE/store half into the next chunk's pipeline.
        outer_ctx = tc.tile_pool(name="ctxp", bufs=2, space="PSUM")
        c_pool = outer_ctx.__enter__()
        outer_rn = tc.tile_pool(name="rnp", bufs=2)
        rn_pool = outer_rn.__enter__()
        outer_ex = tc.tile_pool(name="expp", bufs=3)
        e_pool = outer_ex.__enter__()

        # ---- prolog: window path (-> R) + full-image conv1 ----
        with tc.tile_pool(name="pw1", bufs=1, space="PSUM") as pw1, \
             tc.tile_pool(name="pw2", bufs=2, space="PSUM") as pw2, \
             tc.tile_pool(name="pc1", bufs=2, space="PSUM") as pc1, \
             tc.tile_pool(name="pr", bufs=1, space="PSUM") as pr:
            # window conv1 (both frames, block-diag K=18)
            for (r0, nr) in [(0, 4), (4, 4), (8, 4), (12, 4), (16, 4), (20, 4), (24, 2)]:
                ps = pw1.tile([CH, RPC * W], f32, tag="c1w")
                nc.tensor.matmul(ps[:, 0:nr * W], lhsT=w1t[:, :],
                                 rhs=x9wv[:, r0:r0 + nr, :], start=True, stop=True)
                nc.vector.tensor_scalar(
                    r1wv[:, r0:r0 + nr, 1:97],
                    ps[:, 0:nr * W].rearrange("p (h w) -> p h w", h=nr),
                    b1s[:, :], 0.0, op0=ADD, op1=MAX)
            # halo-row masks (rows 0 and WR1-1 only)
            nc.vector.tensor_mul(r1wv[:, 0:1, :], r1wv[:, 0:1, :], rmv[:, 0:1, :])
            nc.vector.tensor_mul(r1wv[:, WR1 - 1:WR1, :], r1wv[:, WR1 - 1:WR1, :],
                                 rmv[:, 1:2, :])
            # window conv2 -> featw
            for c in range(WR2 // RPC):
                ps = pw2.tile([CH, RPC * W], f32, tag="c2w")
                for t in range(9):
                    dy, dx = t // 3, t % 3
                    nc.tensor.matmul(
                        ps[:, :], lhsT=w2t[:, bass.ts(t, CH)],
                        rhs=r1wv[:, dy + c * RPC:dy + (c + 1) * RPC, dx:dx + W],
                        start=(t == 0), stop=(t == 8))
                nc.vector.tensor_scalar(featw[:, bass.ts(c, RPC * W)], ps[:, :],
                                        b2s[:, :], 0.0, op0=ADD, op1=MAX)
            # R = G^T featw + h
            for (q0, qn) in QCH:
                ps = pr.tile([CH, 512], f32, tag="r")
                nc.tensor.matmul(ps[:, 0:qn], lhsT=gwt[:, :],
                                 rhs=featw[:, bass.ds(q0, qn)], start=True, stop=True)
                nc.vector.tensor_scalar(rsb[:, bass.ds(q0, qn)], ps[:, 0:qn],
                                        hbs[:, :], None, op0=ADD)
            # full-image conv1 -> r1p interior
            for c in range(NRC):
                r0 = RPC * c
                ps = pc1.tile([CH, RPC * W], f32, tag="c1f")
                nc.tensor.matmul(ps[:, :], lhsT=w1t[:, :],
                                 rhs=x9v[:, r0:r0 + RPC, :], start=True, stop=True)
                nc.vector.tensor_scalar(
                    r1v[:, r0 + 1:r0 + RPC + 1, 1:97],
                    ps[:, :].rearrange("p (h w) -> p h w", h=RPC),
                    b1s[:, :], 0.0, op0=ADD, op1=MAX)

        def s_group(sp, g, n_off, nch):
            for j in range(GROUP):
                m = GROUP * g + j
                nc.tensor.matmul(sp[:, j * 512:j * 512 + nch],
                                 lhsT=feat[:, bass.ts(m, MBLK)],
                                 rhs=rsb[:, bass.ds(n_off, nch)],
                                 start=True, stop=True)

        def exp_group(ex, sp, nch):
            if nch == 512:
                nc.scalar.activation(ex[:, :], sp[:, :], EXP, bias=shf[:, :],
                                     scale=SCALE)
            else:
                nc.scalar.activation(
                    ex[:, :].rearrange("p (j x) -> p j x", x=512)[:, :, 0:nch],
                    sp[:, :].rearrange("p (j x) -> p j x", x=512)[:, :, 0:nch],
                    EXP, bias=shf[:, :], scale=SCALE)

        def av_group(ctx_ps, ex, g, nch):
            for j in range(GROUP):
                m = GROUP * g + j
                nc.tensor.matmul(ctx_ps[:, 0:nch],
                                 lhsT=vt1[:, bass.ts(m, 65)],
                                 rhs=ex[:, j * 512:j * 512 + nch],
                                 start=(m == 0), stop=(m == NMB - 1))

        def norm_dve(ctx_ps, nch):
            # copy ctx out of PSUM fast (frees the bank for the next chunk)
            # and recip the rowsum; DVE only, so the PE queue never waits.
            ctxs = rn_pool.tile([65, 512], f32, tag="ctxs")
            nc.vector.tensor_copy(ctxs[:, 0:nch], ctx_ps[:, 0:nch])
            rr = rn_pool.tile([65, 512], f32, tag="rr")
            nc.vector.reciprocal(rr[64:65, 0:nch], ctxs[64:65, 0:nch])
            return ctxs, rr

        def norm_pe(ctxs, rr, n_off, nch):
            # broadcast 1/rowsum across partitions with a K=1 fp32 outer
            # product into a free ctx-pool slot, then scale + v_b + store.
            bc = c_pool.tile([65, 512], f32, tag="ctx")
            nc.tensor.matmul(bc[0:64, 0:nch], lhsT=ones65[64:65, 0:64],
                             rhs=rr[64:65, 0:nch], start=True, stop=True,
                             tile_position=(64, 0))
            nc.vector.tensor_mul(outs[:, bass.ds(n_off, nch)],
                                 ctxs[0:64, 0:nch], bc[0:64, 0:nch])
            nc.vector.tensor_scalar(outs[:, bass.ds(n_off, nch)],
                                    outs[:, bass.ds(n_off, nch)],
                                    vbs[:, :], None, op0=ADD)
            dma(d_out[:, bass.ds(n_off, nch)], outs[:, bass.ds(n_off, nch)])

        # ---- fused producer + attention chunk 0 ----
        # iteration c: conv2 chunk c -> feat, V^T blocks, AV(c-1), S(c), exp(c)
        ctx0 = c_pool.tile([65, 512], f32, tag="ctx", name="ctx0")
        with tc.tile_pool(name="pc2", bufs=2, space="PSUM") as pc2, \
             tc.tile_pool(name="pv", bufs=1, space="PSUM") as pv, \
             tc.tile_pool(name="sp0", bufs=1, space="PSUM") as sp0_pool:
            prev = None
            for c in range(NRC):
                ps = pc2.tile([CH, RPC * W], f32, tag="c2f")
                for t in range(9):
                    dy, dx = t // 3, t % 3
                    nc.tensor.matmul(
                        ps[:, :], lhsT=w2t[:, bass.ts(t, CH)],
                        rhs=r1v[:, dy + c * RPC:dy + (c + 1) * RPC, dx:dx + W],
                        start=(t == 0), stop=(t == 8))
                nc.vector.tensor_scalar(featv[:, bass.ts(c, RPC * W)], ps[:, :],
                                        b2s[:, :], 0.0, op0=ADD, op1=MAX)
                psv = pv.tile([CH, GROUP * 64], f32, tag="v")
                for j in range(GROUP):
                    m = GROUP * c + j
                    nc.tensor.matmul(psv[:, bass.ts(j, 64)],
                                     lhsT=feat[:, bass.ts(m, MBLK)],
                                     rhs=vwt[:, :], start=True, stop=True)
                nc.vector.tensor_copy(
                    vt1v[:, GROUP * c:GROUP * (c + 1), 0:64],
                    psv[:, :].rearrange("p (t c) -> p t c", c=64))
                if prev is not None:
                    av_group(ctx0, prev[0], prev[1], 512)
                sp = sp0_pool.tile([CH, GROUP * 512], f32, tag="sp0")
                s_group(sp, c, 0, 512)
                ex = e_pool.tile([CH, GROUP * 512], b16, tag="ex0")
                exp_group(ex, sp, 512)
                prev = (ex, c)
            av_group(ctx0, prev[0], prev[1], 512)
            pend = norm_dve(ctx0, 512) + (0, 512)

        # ---- remaining query chunks ----
        with tc.tile_pool(name="spp", bufs=2, space="PSUM") as sp_pool:
            n_off = 512
            for nch in CHUNKS[1:]:
                ctx_ps = c_pool.tile([65, 512], f32, tag="ctx")
                prev = None
                for g in range(NMB // GROUP):
                    if prev is not None:
                        av_group(ctx_ps, prev[0], prev[1], nch)
                    sp = sp_pool.tile([CH, GROUP * 512], f32, tag="sp")
                    s_group(sp, g, n_off, nch)
                    ex = e_pool.tile([CH, GROUP * 512], b16, tag="ex")
                    exp_group(ex, sp, nch)
                    if g == 2 and pend is not None:
                        norm_pe(*pend)
                        pend = None
                    prev = (ex, g)
                av_group(ctx_ps, prev[0], prev[1], nch)
                pend = norm_dve(ctx_ps, nch) + (n_off, nch)
                n_off += nch
            norm_pe(*pend)

        outer_ex.__exit__(None, None, None)
        outer_rn.__exit__(None, None, None)
        outer_ctx.__exit__(None, None, None)

        for _f in reversed(_frees):
            _f()

    nc.compile()
    return nc


def _prep_in_maps(inputs):
    x = np.asarray(inputs["neighbor_frames"], np.float32)
    w1 = np.asarray(inputs["enc_w1"], np.float32)
    w2 = np.asarray(inputs["enc_w2"], np.float32)
    qw = np.asarray(inputs["q_w"], np.float32)
    kw = np.asarray(inputs["k_w"], np.float32)
    vw = np.asarray(inputs["v_w"], np.float32)
    b1 = np.asarray(inputs["enc_b1"], np.float32)
    b2 = np.asarray(inputs["enc_b2"], np.float32)
    qb = np.asarray(inputs["q_b"], np.float32)
    vb = np.asarray(inputs["v_b"], np.float32)
    # k_b intentionally unused: it shifts every logit of a query row by the
    # same constant, which softmax cancels exactly.

    xpad = np.zeros((B, NF, HP, HP), np.float32)
    xpad[:, :, 1:97, 1:97] = x

    # per-core query windows: input rows r0-2 .. r0+25 (zero outside image)
    xbig = np.zeros((B, NF, 102, HP), np.float32)   # row i = input row i-3
    xbig[:, :, 3:99, 1:97] = x
    xwin = np.zeros((B, 4, NF, 28, HP), np.float32)
    for q in range(4):
        r0 = q * 24
        xwin[:, q] = xbig[:, :, r0 + 1:r0 + 29, :]
    xwin = xwin.astype(bf16)
    xpad = xpad.astype(bf16)

    # conv1w halo-row masks: plane 0 = window row 0, plane 1 = window row WR1-1
    rmasks = []
    for q in range(4):
        m = np.ones((CH, 2, HP), np.float32)
        if q == 0:
            m[:, 0, :] = 0.0
        if q == 3:
            m[:, 1, :] = 0.0
        rmasks.append(np.ascontiguousarray(m.reshape(CH, 2 * HP)).astype(bf16))

    # conv1 weights, block-diagonal: taps f0 on rows 0-8 -> cols 0-63,
    # taps f1 on rows 9-17 -> cols 64-127 (same weights, frames share encoder)
    taps = w1.reshape(64, 9).T                     # [9, 64]
    w1t = np.zeros((18, CH), np.float32)
    w1t[0:9, 0:64] = taps
    w1t[9:18, 64:128] = taps
    w1t = w1t.astype(bf16)

    # conv2 block-diagonal per tap: [128 cin, tap*128 + cout]
    w2tap = w2.transpose(2, 3, 1, 0).reshape(9, 64, 64)  # [tap, cin, cout]
    w2t = np.zeros((CH, 9 * CH), np.float32)
    for t in range(9):
        w2t[0:64, t * CH:t * CH + 64] = w2tap[t]
        w2t[64:128, t * CH + 64:t * CH + 128] = w2tap[t]
    w2t = w2t.astype(bf16)

    # feat block layout row i = frame*64 + c  <->  reference channel c*2+frame
    perm = np.array([(i % 64) * 2 + i // 64 for i in range(CH)])
    qwt = np.ascontiguousarray(qw[:, perm].T)      # [128, 64]
    kwt = np.ascontiguousarray(kw[:, perm].T)
    vwt = np.ascontiguousarray(vw[:, perm].T).astype(bf16)
    gwt = np.ascontiguousarray(qwt @ kwt.T).astype(bf16)   # lhsT of R = G^T featw
    hb = np.ascontiguousarray((kwt @ qb).reshape(CH, 1))   # [128, 1] f32

    b1c = np.ascontiguousarray(np.concatenate([b1, b1]).reshape(CH, 1))
    b2c = np.ascontiguousarray(np.concatenate([b2, b2]).reshape(CH, 1))
    vbc = np.ascontiguousarray(vb.reshape(64, 1))

    in_maps = []
    for core in range(NCORES):
        b = core // 4
        q = core % 4
        in_maps.append({
            "xpad": np.ascontiguousarray(xpad[b]),
            "xwin": np.ascontiguousarray(xwin[b, q]),
            "rmsk": rmasks[q],
            "w1t": w1t, "w2t": w2t, "gwt": gwt, "vwt": vwt,
            "b1": b1c, "b2": b2c, "hb": hb, "vb": vbc,
        })
    return in_maps


def _install_ntff_shim():
    """Provide antenv.axon_hooks (absent in this image) so
    run_bass_kernel_spmd(trace=True) can capture NTFF profiles through
    libaxon_pjrt's C ABI, and neuter the S3 artifact upload."""
    import sys, types, ctypes, contextlib

    if "antenv.axon_hooks" not in sys.modules:
        mod = types.ModuleType("antenv.axon_hooks")
        mod._hook = None
        mod.set_axon_ntff_profile_hook = lambda h: setattr(mod, "_hook", h)
        mod.get_axon_ntff_profile_hook = lambda: mod._hook
        sys.modules["antenv.axon_hooks"] = mod

        lib = ctypes.CDLL("/opt/axon/libaxon_pjrt.so")
        if hasattr(lib, "axon_start_nrt_profile"):
            lib.axon_start_nrt_profile.argtypes = [
                ctypes.POINTER(ctypes.c_int64), ctypes.c_size_t]
            lib.axon_start_nrt_profile.restype = ctypes.c_int64
            lib.axon_stop_nrt_profile.argtypes = [ctypes.c_char_p]
            lib.axon_stop_nrt_profile.restype = ctypes.c_int64

            @contextlib.contextmanager
            def _hook(output_dir, device_ids):
                import jax
                jax.devices()
                if device_ids:
                    ids = (ctypes.c_int64 * len(device_ids))(*device_ids)
                    rc = lib.axon_start_nrt_profile(ids, len(device_ids))
                else:
                    rc = lib.axon_start_nrt_profile(None, 0)
                if rc != 0:
                    raise RuntimeError(f"axon_start_nrt_profile rc={rc}")
                try:
                    yield
                finally:
                    n = lib.axon_stop_nrt_profile(str(output_dir).encode())
                    print(f"ntff profile: {n} file(s) -> {output_dir}")

            mod.set_axon_ntff_profile_hook(_hook)

    import concourse.bass_utils as _bu
    _bu.upload_artifacts = lambda tmpdir: tmpdir


def kernel(**inputs):
    global _COMPILED, LAST_RESULTS
    from concourse.bass_utils import run_bass_kernel_spmd

    if _COMPILED is None:
        _COMPILED = _build_nc()
    nc = _COMPILED

    in_maps = _prep_in_maps(inputs)
    trace = bool(int(__import__("os").environ.get("CA_TRACE", "0")))
    if trace:
        _install_ntff_shim()
    res = run_bass_kernel_spmd(nc, in_maps, core_ids=list(range(NCORES)),
                               trace=trace)
    LAST_RESULTS = res

    out = np.zeros((B, 64, T), np.float32)
    for core in range(NCORES):
        b = core // 4
        q = core % 4
        out[b, :, q * NB:(q + 1) * NB] = res.results[core]["out"]
    return out.reshape(B, 64, H, W)


# revision 23
# speedup vs baseline: 1.2116x; 1.0069x over previous
"""Trainium2 Bass kernel for nn_ContextAttention (dense_transformer).

Reference model:
  neighbor_frames [2,2,96,96] -> per-frame conv3x3(1->64)+relu -> conv3x3(64->64)+relu
  feat [B, 128, 9216], Q/K/V 1x1 projections (64 out ch),
  attn = softmax(Q^T K / 8) [B, 9216, 9216], out = (attn @ V^T)^T -> [B, 64, 96, 96]

Distribution: 8 cores = 2 batches x 4 query-row blocks of 2304 tokens each.
Every core computes its batch's encoder feat (cheap vs attention) plus a
flash-style attention slice in S^T layout (key token m on partitions):

  R      = G^T feat_win + h        G = Wk Wq^T (feat basis), h = Wk q_b
  S^T    = feat_blk^T R            (PE, full K=128 contraction; q_b exact via h,
                                    k_b drops out of softmax exactly)
  expS   = exp(0.125 S^T - 4)      (ScalarE; softmax shift invariance)
  [ctx; rowsum] = sum_m expS[m,n] * VT1[m,:]   (PE; VT1 = [V^T | ones col])
  out    = ctx / rowsum + v_b

Both conv layers run the two frames as one block-diagonal matmul (f0 channels
on partitions 0-63, f1 on 64-127), so conv outputs land directly in the packed
feat layout with no cross-partition staging.  The whole kernel is software-
pipelined: attention query-chunk 0 (S/exp/AV) interleaves with the conv2/V
producer loop, group g consuming exactly the 3 m-blocks conv2 chunk g just
produced; AV lags one iteration so the PE never stalls on ScalarE's exp.

SPMD note: all 8 cores run one program; the only per-core knob is data.  The
per-core query window is realized by recomputing the two convs on a small
host-sliced window of the input image (xwin); out-of-image conv1 halo rows are
zeroed with a host mask to reproduce the reference's zero padding exactly.
"""

import numpy as np
import ml_dtypes

bf16 = ml_dtypes.bfloat16

B = 2
NF = 2
H = W = 96
HP = 98
T = H * W            # 9216
CH = 128
NB = T // 4          # 2304 query rows per core
NCORES = 8
MBLK = 128
NMB = T // MBLK      # 72
GROUP = 3            # m-blocks per exp group (3 PSUM banks)
CHUNKS = [512, 512, 512, 512, 256]
SHIFT = 4.0
SCALE = 0.125
WR1 = 26             # window conv1 output rows
WR2 = 24             # window conv2 output rows
RPC = 4              # image rows per conv chunk
NRC = H // RPC       # 24 full-image chunks; also the fused-loop length

_COMPILED = None
LAST_RESULTS = None


def _build_nc():
    import concourse.bass as bass
    import concourse.tile as tile
    from concourse import bacc, mybir

    f32 = mybir.dt.float32
    b16 = mybir.dt.bfloat16
    EXP = mybir.ActivationFunctionType.Exp
    ADD = mybir.AluOpType.add
    MAX = mybir.AluOpType.max

    nc = bacc.Bacc("TRN2", target_bir_lowering=False, debug=False,
                   enable_asserts=False, num_devices=NCORES)

    d_xpad = nc.dram_tensor("xpad", [NF, HP, HP], b16, kind="ExternalInput").ap()
    d_xwin = nc.dram_tensor("xwin", [NF, 28, HP], b16, kind="ExternalInput").ap()
    d_w1t = nc.dram_tensor("w1t", [18, CH], b16, kind="ExternalInput").ap()
    d_w2t = nc.dram_tensor("w2t", [CH, 9 * CH], b16, kind="ExternalInput").ap()
    d_gwt = nc.dram_tensor("gwt", [CH, CH], b16, kind="ExternalInput").ap()
    d_vwt = nc.dram_tensor("vwt", [CH, 64], b16, kind="ExternalInput").ap()
    d_b1 = nc.dram_tensor("b1", [CH, 1], f32, kind="ExternalInput").ap()
    d_b2 = nc.dram_tensor("b2", [CH, 1], f32, kind="ExternalInput").ap()
    d_hb = nc.dram_tensor("hb", [CH, 1], f32, kind="ExternalInput").ap()
    d_vb = nc.dram_tensor("vb", [64, 1], f32, kind="ExternalInput").ap()
    d_rmsk = nc.dram_tensor("rmsk", [CH, 2 * HP], b16, kind="ExternalInput").ap()
    d_out = nc.dram_tensor("out", [64, NB], f32, kind="ExternalOutput").ap()

    with tile.TileContext(nc) as tc:
        _frees = []

        def _keep(pair):
            _frees.append(pair[1])
            return pair[0]

        w1t = _keep(tc.tile([18, CH], b16, name="w1t_sb"))
        w2t = _keep(tc.tile([CH, 9 * CH], b16, name="w2t_sb"))
        gwt = _keep(tc.tile([CH, CH], b16, name="gwt_sb"))
        vwt = _keep(tc.tile([CH, 64], b16, name="vwt_sb"))
        b1s = _keep(tc.tile([CH, 1], f32, name="b1_sb"))
        b2s = _keep(tc.tile([CH, 1], f32, name="b2_sb"))
        hbs = _keep(tc.tile([CH, 1], f32, name="hb_sb"))
        vbs = _keep(tc.tile([64, 1], f32, name="vb_sb"))
        rmsk = _keep(tc.tile([CH, 2 * HP], b16, name="rmsk_sb"))
        x9 = _keep(tc.tile([18, T], b16, name="x9_sb"))          # f0 @ p0-8, f1 @ p9-17
        x9w = _keep(tc.tile([18, WR1 * W], b16, name="x9w_sb"))
        r1p = _keep(tc.tile([CH, HP * HP], b16, name="r1pad_sb"))  # f0 @ p0-63, f1 @ p64-127
        r1wp = _keep(tc.tile([CH, WR1 * HP], b16, name="r1wp_sb"))
        feat = _keep(tc.tile([CH, T], b16, name="feat_sb"))
        featw = _keep(tc.tile([CH, NB], b16, name="featw_sb"))
        rsb = _keep(tc.tile([CH, NB], b16, name="r_sb"))
        vt1 = _keep(tc.tile([CH, NMB * 65], b16, name="vt1_sb"))
        outs = _keep(tc.tile([64, NB], f32, name="out_sb"))
        shf = _keep(tc.tile([CH, 1], f32, name="shift_sb"))
        ones65 = _keep(tc.tile([65, 64], f32, name="ones65_sb"))

        dma = nc.sync.dma_start
        dma2 = nc.scalar.dma_start   # second DGE queue (scalar idle in prolog)

        # ---- load inputs: conv1w's deps split across both queues, rest after ----
        dma(w1t[:, :], d_w1t)
        dma(b1s[:, :], d_b1)
        for t in range(9):
            dy, dx = t // 3, t % 3
            dma(x9w[t:t + 1, :].rearrange("p (h w) -> p h w", h=WR1),
                d_xwin[0:1, dy:dy + WR1, dx:dx + W])
            dma2(x9w[9 + t:10 + t, :].rearrange("p (h w) -> p h w", h=WR1),
                 d_xwin[1:2, dy:dy + WR1, dx:dx + W])
        dma2(rmsk[:, :], d_rmsk)
        dma(w2t[:, :], d_w2t)
        dma(b2s[:, :], d_b2)
        dma(gwt[:, :], d_gwt)
        dma(hbs[:, :], d_hb)
        dma(vwt[:, :], d_vwt)
        dma(vbs[:, :], d_vb)
        for t in range(9):
            dy, dx = t // 3, t % 3
            dma(x9[t:t + 1, :].rearrange("p (h w) -> p h w", h=H),
                d_xpad[0:1, dy:dy + H, dx:dx + W])
            dma2(x9[9 + t:10 + t, :].rearrange("p (h w) -> p h w", h=H),
                 d_xpad[1:2, dy:dy + H, dx:dx + W])

        nc.vector.memset(shf[:, :], -SHIFT)
        nc.vector.memset(ones65[:, :], 1.0)
        # padded-conv1-output borders (interiors fully written below)
        r1v = r1p[:, :].rearrange("p (h w) -> p h w", h=HP)
        nc.vector.memset(r1v[:, 0:1, :], 0.0)
        nc.vector.memset(r1v[:, HP - 1:HP, :], 0.0)
        nc.vector.memset(r1v[:, :, 0:1], 0.0)
        nc.vector.memset(r1v[:, :, HP - 1:HP], 0.0)
        r1wv = r1wp[:, :].rearrange("p (h w) -> p h w", h=WR1)
        nc.vector.memset(r1wv[:, :, 0:1], 0.0)
        nc.vector.memset(r1wv[:, :, HP - 1:HP], 0.0)
        vt1v = vt1[:, :].rearrange("p (t c) -> p t c", c=65)
        nc.vector.memset(vt1v[:, :, 64:65], 1.0)

        x9v = x9[:, :].rearrange("p (h w) -> p h w", h=H)
        x9wv = x9w[:, :].rearrange("p (h w) -> p h w", h=WR1)
        featv = feat[:, :]
        rmv = rmsk[:, :].rearrange("p (h w) -> p h w", h=2)
        QCH = [(0, 512), (512, 512), (1024, 512), (1536, 512), (2048, 256)]

        # ctx pool (2 banks) spans all phases so chunk normalizes can defer
        # their PE/store half into the next chunk's pipeline.
        outer_ctx = tc.tile_pool(name="ctxp", bufs=2, space="PSUM")
        c_pool = outer_ctx.__enter__()
        outer_rn = tc.tile_pool(name="rnp", bufs=2)
        rn_pool = outer_rn.__enter__()
        outer_ex = tc.tile_pool(name="expp", bufs=3)
        e_pool = outer_ex.__enter__()

        # ---- prolog: window path (-> R) + full-image conv1 ----
        with tc.tile_pool(name="pw1", bufs=1, space="PSUM") as pw1, \
             tc.tile_pool(name="pw2", bufs=2, space="PSUM") as pw2, \
             tc.tile_pool(name="pc1", bufs=2, space="PSUM") as pc1, \
             tc.tile_pool(name="pr", bufs=1, space="PSUM") as pr:
            # window conv1 (both frames, block-diag K=18)
            for (r0, nr) in [(0, 4), (4, 4), (8, 4), (12, 4), (16, 4), (20, 4), (24, 2)]:
                ps = pw1.tile([CH, RPC * W], f32, tag="c1w")
                nc.tensor.matmul(ps[:, 0:nr * W], lhsT=w1t[:, :],
                                 rhs=x9wv[:, r0:r0 + nr, :], start=True, stop=True)
                nc.vector.tensor_scalar(
                    r1wv[:, r0:r0 + nr, 1:97],
                    ps[:, 0:nr * W].rearrange("p (h w) -> p h w", h=nr),
                    b1s[:, :], 0.0, op0=ADD, op1=MAX)
            # halo-row masks (rows 0 and WR1-1 only)
            nc.vector.tensor_mul(r1wv[:, 0:1, :], r1wv[:, 0:1, :], rmv[:, 0:1, :])
            nc.vector.tensor_mul(r1wv[:, WR1 - 1:WR1, :], r1wv[:, WR1 - 1:WR1, :],
                                 rmv[:, 1:2, :])
            # window conv2 -> featw
            for c in range(WR2 // RPC):
                ps = pw2.tile([CH, RPC * W], f32, tag="c2w")
                for t in range(9):
                    dy, dx = t // 3, t % 3
                    nc.tensor.matmul(
                        ps[:, :], lhsT=w2t[:, bass.ts(t, CH)],
                        rhs=r1wv[:, dy + c * RPC:dy + (c + 1) * RPC, dx:dx + W],
                        start=(t == 0), stop=(t == 8))
                nc.vector.tensor_scalar(featw[:, bass.ts(c, RPC * W)], ps[:, :],
                                        b2s[:, :], 0.0, op0=ADD, op1=MAX)
            # R = G^T featw + h
            for (q0, qn) in QCH:
                ps = pr.tile([CH, 512], f32, tag="r")
                nc.tensor.matmul(ps[:, 0:qn], lhsT=gwt[:, :],
                                 rhs=featw[:, bass.ds(q0, qn)], start=True, stop=True)
                nc.vector.tensor_scalar(rsb[:, bass.ds(q0, qn)], ps[:, 0:qn],
                                        hbs[:, :], None, op0=ADD)
            # full-image conv1 -> r1p interior
            for c in range(NRC):
                r0 = RPC * c
                ps = pc1.tile([CH, RPC * W], f32, tag="c1f")
                nc.tensor.matmul(ps[:, :], lhsT=w1t[:, :],
                                 rhs=x9v[:, r0:r0 + RPC, :], start=True, stop=True)
                nc.vector.tensor_scalar(
                    r1v[:, r0 + 1:r0 + RPC + 1, 1:97],
                    ps[:, :].rearrange("p (h w) -> p h w", h=RPC),
                    b1s[:, :], 0.0, op0=ADD, op1=MAX)

        def s_group(sp, g, n_off, nch):
            for j in range(GROUP):
                m = GROUP * g + j
                nc.tensor.matmul(sp[:, j * 512:j * 512 + nch],
                                 lhsT=feat[:, bass.ts(m, MBLK)],
                                 rhs=rsb[:, bass.ds(n_off, nch)],
                                 start=True, stop=True)

        def exp_group(ex, sp, nch):
            if nch == 512:
                nc.scalar.activation(ex[:, :], sp[:, :], EXP, bias=shf[:, :],
                                     scale=SCALE)
            else:
                nc.scalar.activation(
                    ex[:, :].rearrange("p (j x) -> p j x", x=512)[:, :, 0:nch],
                    sp[:, :].rearrange("p (j x) -> p j x", x=512)[:, :, 0:nch],
                    EXP, bias=shf[:, :], scale=SCALE)

        def av_group(ctx_ps, ex, g, nch):
            for j in range(GROUP):
                m = GROUP * g + j
                nc.tensor.matmul(ctx_ps[:, 0:nch],
                                 lhsT=vt1[:, bass.ts(m, 65)],
                                 rhs=ex[:, j * 512:j * 512 + nch],
                                 start=(m == 0), stop=(m == NMB - 1))

        def norm_dve(ctx_ps, nch):
            # copy ctx out of PSUM fast (frees the bank for the next chunk)
            # and recip the rowsum; DVE only, so the PE queue never waits.
            ctxs = rn_pool.tile([65, 512], f32, tag="ctxs")
            nc.vector.tensor_copy(ctxs[:, 0:nch], ctx_ps[:, 0:nch])
            rr = rn_pool.tile([65, 512], f32, tag="rr")
            nc.vector.reciprocal(rr[64:65, 0:nch], ctxs[64:65, 0:nch])
            return ctxs, rr

        def norm_pe(ctxs, rr, n_off, nch):
            # broadcast 1/rowsum across partitions with a K=1 fp32 outer
            # product into a free ctx-pool slot, then scale + v_b + store.
            bc = c_pool.tile([65, 512], f32, tag="ctx")
            nc.tensor.matmul(bc[0:64, 0:nch], lhsT=ones65[64:65, 0:64],
                             rhs=rr[64:65, 0:nch], start=True, stop=True,
                             tile_position=(64, 0))
            nc.vector.tensor_mul(outs[:, bass.ds(n_off, nch)],
                                 ctxs[0:64, 0:nch], bc[0:64, 0:nch])
            nc.vector.tensor_scalar(outs[:, bass.ds(n_off, nch)],
                                    outs[:, bass.ds(n_off, nch)],
                                    vbs[:, :], None, op0=ADD)
            dma(d_out[:, bass.ds(n_off, nch)], outs[:, bass.ds(n_off, nch)])

        # ---- fused producer + attention chunk 0 ----
        # iteration c: conv2 chunk c -> feat, V^T blocks, AV(c-1), S(c), exp(c)
        ctx0 = c_pool.tile([65, 512], f32, tag="ctx", name="ctx0")
        with tc.tile_pool(name="pc2", bufs=2, space="PSUM") as pc2, \
             tc.tile_pool(name="pv", bufs=1, space="PSUM") as pv, \
             tc.tile_pool(name="sp0", bufs=1, space="PSUM") as sp0_pool:
            # V/S/exp lag conv2 by one chunk so the PE never waits on the
            # DVE relu of the chunk it is about to consume.
            prev = None
            for c in range(NRC + 1):
                if c < NRC:
                    ps = pc2.tile([CH, RPC * W], f32, tag="c2f")
                    for t in range(9):
                        dy, dx = t // 3, t % 3
                        nc.tensor.matmul(
                            ps[:, :], lhsT=w2t[:, bass.ts(t, CH)],
                            rhs=r1v[:, dy + c * RPC:dy + (c + 1) * RPC, dx:dx + W],
                            start=(t == 0), stop=(t == 8))
                    nc.vector.tensor_scalar(featv[:, bass.ts(c, RPC * W)],
                                            ps[:, :], b2s[:, :], 0.0,
                                            op0=ADD, op1=MAX)
                if c == 0:
                    continue
                cc = c - 1
                if prev is not None:
                    av_group(ctx0, prev[0], prev[1], 512)
                psv = pv.tile([CH, GROUP * 64], f32, tag="v")
                for j in range(GROUP):
                    m = GROUP * cc + j
                    nc.tensor.matmul(psv[:, bass.ts(j, 64)],
                                     lhsT=feat[:, bass.ts(m, MBLK)],
                                     rhs=vwt[:, :], start=True, stop=True)
                nc.vector.tensor_copy(
                    vt1v[:, GROUP * cc:GROUP * (cc + 1), 0:64],
                    psv[:, :].rearrange("p (t c) -> p t c", c=64))
                sp = sp0_pool.tile([CH, GROUP * 512], f32, tag="sp0")
                s_group(sp, cc, 0, 512)
                ex = e_pool.tile([CH, GROUP * 512], b16, tag="ex0")
                exp_group(ex, sp, 512)
                prev = (ex, cc)
            av_group(ctx0, prev[0], prev[1], 512)
            pend = norm_dve(ctx0, 512) + (0, 512)

        # ---- remaining query chunks ----
        with tc.tile_pool(name="spp", bufs=2, space="PSUM") as sp_pool:
            n_off = 512
            for nch in CHUNKS[1:]:
                ctx_ps = c_pool.tile([65, 512], f32, tag="ctx")
                prev = None
                for g in range(NMB // GROUP):
                    if prev is not None:
                        av_group(ctx_ps, prev[0], prev[1], nch)
                    sp = sp_pool.tile([CH, GROUP * 512], f32, tag="sp")
                    s_group(sp, g, n_off, nch)
                    ex = e_pool.tile([CH, GROUP * 512], b16, tag="ex")
                    exp_group(ex, sp, nch)
                    if g == 2 and pend is not None:
                        norm_pe(*pend)
                        pend = None
                    prev = (ex, g)
                av_group(ctx_ps, prev[0], prev[1], nch)
                pend = norm_dve(ctx_ps, nch) + (n_off, nch)
                n_off += nch
            norm_pe(*pend)

        outer_ex.__exit__(None, None, None)
        outer_rn.__exit__(None, None, None)
        outer_ctx.__exit__(None, None, None)

        for _f in reversed(_frees):
            _f()

    nc.compile()
    return nc


def _prep_in_maps(inputs):
    x = np.asarray(inputs["neighbor_frames"], np.float32)
    w1 = np.asarray(inputs["enc_w1"], np.float32)
    w2 = np.asarray(inputs["enc_w2"], np.float32)
    qw = np.asarray(inputs["q_w"], np.float32)
    kw = np.asarray(inputs["k_w"], np.float32)
    vw = np.asarray(inputs["v_w"], np.float32)
    b1 = np.asarray(inputs["enc_b1"], np.float32)
    b2 = np.asarray(inputs["enc_b2"], np.float32)
    qb = np.asarray(inputs["q_b"], np.float32)
    vb = np.asarray(inputs["v_b"], np.float32)
    # k_b intentionally unused: it shifts every logit of a query row by the
    # same constant, which softmax cancels exactly.

    xpad = np.zeros((B, NF, HP, HP), np.float32)
    xpad[:, :, 1:97, 1:97] = x

    # per-core query windows: input rows r0-2 .. r0+25 (zero outside image)
    xbig = np.zeros((B, NF, 102, HP), np.float32)   # row i = input row i-3
    xbig[:, :, 3:99, 1:97] = x
    xwin = np.zeros((B, 4, NF, 28, HP), np.float32)
    for q in range(4):
        r0 = q * 24
        xwin[:, q] = xbig[:, :, r0 + 1:r0 + 29, :]
    xwin = xwin.astype(bf16)
    xpad = xpad.astype(bf16)

    # conv1w halo-row masks: plane 0 = window row 0, plane 1 = window row WR1-1
    rmasks = []
    for q in range(4):
        m = np.ones((CH, 2, HP), np.float32)
        if q == 0:
            m[:, 0, :] = 0.0
        if q == 3:
            m[:, 1, :] = 0.0
        rmasks.append(np.ascontiguousarray(m.reshape(CH, 2 * HP)).astype(bf16))

    # conv1 weights, block-diagonal: taps f0 on rows 0-8 -> cols 0-63,
    # taps f1 on rows 9-17 -> cols 64-127 (same weights, frames share encoder)
    taps = w1.reshape(64, 9).T                     # [9, 64]
    w1t = np.zeros((18, CH), np.float32)
    w1t[0:9, 0:64] = taps
    w1t[9:18, 64:128] = taps
    w1t = w1t.astype(bf16)

    # conv2 block-diagonal per tap: [128 cin, tap*128 + cout]
    w2tap = w2.transpose(2, 3, 1, 0).reshape(9, 64, 64)  # [tap, cin, cout]
    w2t = np.zeros((CH, 9 * CH), np.float32)
    for t in range(9):
        w2t[0:64, t * CH:t * CH + 64] = w2tap[t]
        w2t[64:128, t * CH + 64:t * CH + 128] = w2tap[t]
    w2t = w2t.astype(bf16)

    # feat block layout row i = frame*64 + c  <->  reference channel c*2+frame
    perm = np.array([(i % 64) * 2 + i // 64 for i in range(CH)])
    qwt = np.ascontiguousarray(qw[:, perm].T)      # [128, 64]
    kwt = np.ascontiguousarray(kw[:, perm].T)
    vwt = np.ascontiguousarray(vw[:, perm].T).astype(bf16)
    gwt = np.ascontiguousarray(qwt @ kwt.T).astype(bf16)   # lhsT of R = G^T featw
    hb = np.ascontiguousarray((kwt @ qb).reshape(CH, 1))   # [128, 1] f32

    b1c = np.ascontiguousarray(np.concatenate([b1, b1]).reshape(CH, 1))
    b2c = np.ascontiguousarray(np.concatenate([b2, b2]).reshape(CH, 1))
    vbc = np.ascontiguousarray(vb.reshape(64, 1))

    in_maps = []
    for core in range(NCORES):
        b = core // 4
        q = core % 4
        in_maps.append({
            "xpad": np.ascontiguousarray(xpad[b]),
            "xwin": np.ascontiguousarray(xwin[b, q]),
            "rmsk": rmasks[q],
            "w1t": w1t, "w2t": w2t, "gwt": gwt, "vwt": vwt,
            "b1": b1c, "b2": b2c, "hb": hb, "vb": vbc,
        })
    return in_maps


def _install_ntff_shim():
    """Provide antenv.axon_hooks (absent in this image) so
    run_bass_kernel_spmd(trace=True) can capture NTFF profiles through
    libaxon_pjrt's C ABI, and neuter the S3 artifact upload."""
    import sys, types, ctypes, contextlib

    if "antenv.axon_hooks" not in sys.modules:
        mod = types.ModuleType("antenv.axon_hooks")
        mod._hook = None
        mod.set_axon_ntff_profile_hook = lambda h: setattr(mod, "_hook", h)
        mod.get_axon_ntff_profile_hook = lambda: mod._hook
        sys.modules["antenv.axon_hooks"] = mod

        lib = ctypes.CDLL("/opt/axon/libaxon_pjrt.so")
        if hasattr(lib, "axon_start_nrt_profile"):
            lib.axon_start_nrt_profile.argtypes = [
                ctypes.POINTER(ctypes.c_int64), ctypes.c_size_t]
            lib.axon_start_nrt_profile.restype = ctypes.c_int64
            lib.axon_stop_nrt_profile.argtypes = [ctypes.c_char_p]
            lib.axon_stop_nrt_profile.restype = ctypes.c_int64

            @contextlib.contextmanager
            def _hook(output_dir, device_ids):
                import jax
                jax.devices()
                if device_ids:
                    ids = (ctypes.c_int64 * len(device_ids))(*device_ids)
                    rc = lib.axon_start_nrt_profile(ids, len(device_ids))
                else:
                    rc = lib.axon_start_nrt_profile(None, 0)
                if rc != 0:
                    raise RuntimeError(f"axon_start_nrt_profile rc={rc}")
                try:
                    yield
                finally:
                    n = lib.axon_stop_nrt_profile(str(output_dir).encode())
                    print(f"ntff profile: {n} file(s) -> {output_dir}")

            mod.set_axon_ntff_profile_hook(_hook)

    import concourse.bass_utils as _bu
    _bu.upload_artifacts = lambda tmpdir: tmpdir


def kernel(**inputs):
    global _COMPILED, LAST_RESULTS
    from concourse.bass_utils import run_bass_kernel_spmd

    if _COMPILED is None:
        _COMPILED = _build_nc()
    nc = _COMPILED

    in_maps = _prep_in_maps(inputs)
    trace = bool(int(__import__("os").environ.get("CA_TRACE", "0")))
    if trace:
        _install_ntff_shim()
    res = run_bass_kernel_spmd(nc, in_maps, core_ids=list(range(NCORES)),
                               trace=trace)
    LAST_RESULTS = res

    out = np.zeros((B, 64, T), np.float32)
    for core in range(NCORES):
        b = core // 4
        q = core % 4
        out[b, :, q * NB:(q + 1) * NB] = res.results[core]["out"]
    return out.reshape(B, 64, H, W)
